# revision 30
# baseline (speedup 1.0000x reference)
"""CrossAttentionBlock kernel for 8 Trainium2 NeuronCores.

Sharding: 16 (batch, head) pairs -> 8 cores, each core owns one batch b and
two heads (2*hp, 2*hp+1).  Per core:
  qT/kT = (Wq/Wk slice)^T-projection of condition[b]   [128=2*64 d, 4096 t]
  v     = x[b] @ Wv slice                               [4096 j, 128 dv]
  S^T   = kT^T-slices @ qT  (per head, row-packed on the PE)
  P     = exp(S^T - 8)  (ScalarE, PSUM->SBUF, bf16)
  out^T = v^T @ P^T  accumulated over j (col-packed 2 heads), Z via ones-matmul
  final = (out^T / Z)^T @ Wu slice  -> partial [4096, 512] fp32
Host sums the 4 per-batch partials and adds b_u.

The attention runs as one flat software-pipelined stream over all
(i-block, j-chunk) pairs: scores+exp for chunk n issue ahead of the PV/Z
consumption of chunk n-LAG, so the in-order PE queue never stalls behind
the ScalarE exp, and ScalarE never gaps at i-block boundaries.
"""

import numpy as np
import ml_dtypes

B, T, C = 2, 4096, 512
H, DH = 8, 64
COND = 512
SCALE = (DH // H) ** -0.5  # faithful to reference: 8**-0.5
NCORES = 8
DV = 2 * DH          # per-core head-pair width = 128
CK = COND // 128     # 4 contraction chunks
TJ = T // 128        # 32 key chunks
IB = 512             # query block
NIB = T // IB        # 8
LAG = 3              # chunks the PV/Z consumption trails the scores/exp
EXP_BIAS = -8.0      # constant shift inside exp; cancels in the softmax ratio

# ---- exp work split: ScalarE (exact LUT exp) vs DVE (Schraudolph bits) ----
# each chunk's exp is split by i-range and runs on BOTH engines concurrently:
# ScalarE does i < EXP_SPLIT_I exactly, the DVE does i >= EXP_SPLIT_I via the
# Schraudolph bits trick.  This halves the exp latency on the critical
# scores(n) -> exp(n) -> scores(n+2) PSUM-slot-ring chain.
EXP_SPLIT_I = 288
_LOG2E = 1.4426950408889634
_SIGMA = 0.0579848          # Schraudolph minimax offset
# y = S*EXPC0 + EXPC1, written as int16; those bits ARE bf16(exp(S-8)) approx
EXP_C0 = 128.0 * _LOG2E
EXP_C1 = 128.0 * (127.0 - _SIGMA + EXP_BIAS * _LOG2E)
EXP_CLAMP = 30000.0         # < int16 max; e^~82 — far above any real score

_BUILT = None
_EXP_OP = None


def _get_exp_op():
    """Register (once) a custom DVE op computing Schraudolph exp:
    out_bits16 = clamp(round(x*C0 + C1), 0, C2); writing those int16 bits
    into a bf16-viewed tile yields ~3%-accurate exp(x - 8) at 1 elem/cycle
    on the otherwise-idle Vector engine."""
    global _EXP_OP
    if _EXP_OP is not None:
        return _EXP_OP
    import concourse.dve_ops as dve_ops
    from concourse.dve_ops import DveOp
    from concourse.dve_spec import Spec, Src0, C0, C1, C2, lower, minn, relu
    from concourse.dve_uop import DveOpSpec

    NAME = "EXP2_SCHRAUDOLPH_ANT"
    for op in dve_ops.OPS:
        if op.name == NAME:
            _EXP_OP = op
            return op
    spec = Spec(
        body=minn(relu(Src0 * C0 + C1), C2),
        reference=lambda in0, in1, s0, s1, imm2: np.minimum(
            np.maximum(in0 * s0 + s1, 0.0), imm2
        ),
    )
    row = dve_ops._CUSTOM_DVE_ROW_BASE + len(dve_ops.OPS)
    shas = {}
    for ver in ("v3", "v4"):
        try:
            sp = DveOpSpec(name=NAME, opcode=row, uops=lower(spec, ver=ver))
            shas[ver] = sp.sha(ver)
        except Exception:
            pass
    op = DveOp(NAME, spec, subdim=False, uops_sha=shas)
    dve_ops.OPS.append(op)
    dve_ops.CUSTOM_DVE_SPECS[NAME] = spec
    dve_ops._SUB_OPCODE_FOR_NAME[NAME] = row
    _EXP_OP = op
    return op


def _build_nc():
    import concourse.bass as bass  # noqa: F401
    import concourse.tile as tile
    from concourse import bacc, mybir

    f32 = mybir.dt.float32
    bf16 = mybir.dt.bfloat16
    i16 = mybir.dt.int16
    exp_op = _get_exp_op()

    nc = bacc.Bacc(None)
    condT_d = nc.declare_dram_parameter("condT", [COND, T], bf16, isOutput=False)
    xT_d = nc.declare_dram_parameter("xT", [C, T], bf16, isOutput=False)
    Wq_d = nc.declare_dram_parameter("Wq", [COND, DV], bf16, isOutput=False)
    Wk_d = nc.declare_dram_parameter("Wk", [COND, DV], bf16, isOutput=False)
    Wv_d = nc.declare_dram_parameter("Wv", [C, DV], bf16, isOutput=False)
    Wu_d = nc.declare_dram_parameter("Wu", [DV, C], bf16, isOutput=False)
    sel_d = nc.declare_dram_parameter("sel", [128, 128], f32, isOutput=False)
    out_d = nc.declare_dram_parameter("out", [T, C], f32, isOutput=True)

    Exp = mybir.ActivationFunctionType.Exp

    with tile.TileContext(nc) as tc:
        with (
            tc.tile_pool(name="persist", bufs=1) as persist,
            tc.tile_pool(name="work", bufs=3) as work,
            tc.tile_pool(name="outsb", bufs=3) as outsb,
            tc.tile_pool(name="pt_pool", bufs=LAG + 2) as pt_pool,
            tc.tile_pool(name="stage_ps", bufs=2, space="PSUM") as stage_ps,
            tc.tile_pool(name="pv_ps", bufs=2, space="PSUM") as pv_ps,
            tc.tile_pool(name="zb_ps", bufs=2, space="PSUM") as zb_ps,
        ):
            # ---------------- load inputs (weights first, then sliced) -----
            Wq_sb = persist.tile([128, CK, DV], bf16)
            nc.sync.dma_start(Wq_sb, Wq_d.rearrange("(co ci) d -> ci co d", ci=128))
            Wk_sb = persist.tile([128, CK, DV], bf16)
            nc.sync.dma_start(Wk_sb, Wk_d.rearrange("(co ci) d -> ci co d", ci=128))
            Wv_sb = persist.tile([128, CK, DV], bf16)
            nc.sync.dma_start(Wv_sb, Wv_d.rearrange("(co ci) d -> ci co d", ci=128))
            Wu_sb = persist.tile([128, C], bf16)
            nc.sync.dma_start(Wu_sb, Wu_d[:, :])
            ones_sb = persist.tile([128, 64], bf16)
            nc.vector.memset(ones_sb, 1.0)
            ebias_sb = persist.tile([128, 1], f32)
            nc.vector.memset(ebias_sb, EXP_BIAS)
            sel_sb = persist.tile([128, 128], f32)
            nc.sync.dma_start(sel_sb, sel_d[:, :])

            condT_r = condT_d.rearrange("(co ci) t -> ci co t", ci=128)
            condT_sb = persist.tile([128, CK, T], bf16)
            xT_r = xT_d.rearrange("(co ci) t -> ci co t", ci=128)
            xT_sb = persist.tile([128, CK, T], bf16)
            for ts in range(T // 512):
                sl = slice(ts * 512, (ts + 1) * 512)
                nc.sync.dma_start(xT_sb[:, :, sl], xT_r[:, :, sl])
                nc.sync.dma_start(condT_sb[:, :, sl], condT_r[:, :, sl])

            qT_sb = persist.tile([128, T], bf16)  # partitions 0:64 h0 d, 64:128 h1
            kT_sb = persist.tile([128, T], bf16)
            v_sb = persist.tile([128, TJ, DV], bf16)  # [j_inner, j_outer, dv]

            def qk_proj(ts, W_sb, out_sb):
                # one 512-wide t-slice of the q^T (or k^T) projection
                sl = slice(ts * 512, (ts + 1) * 512)
                p_ps = stage_ps.tile(
                    [128, 512], f32, tag="stage", name=f"pj_{out_sb.tensor.name}_{ts}"
                )
                for ck in range(CK):
                    nc.tensor.matmul(
                        p_ps,
                        lhsT=W_sb[:, ck, :],
                        rhs=condT_sb[:, ck, sl],
                        start=(ck == 0),
                        stop=(ck == CK - 1),
                    )
                nc.vector.tensor_copy(out_sb[:, sl], p_ps)

            # ---------------- flat pipelined attention ----------------
            pvs = {}
            zbs = {}
            pts = {}

            def v_proj_chunk(tj):
                # v[j, dv] for one 128-row j chunk; borrows a zb-pool slot
                v_psum = zb_ps.tile([128, 512], f32, tag="zb", name=f"v_psum_{tj}")
                for ck in range(CK):
                    nc.tensor.matmul(
                        v_psum[:, 0:DV],
                        lhsT=xT_sb[:, ck, tj * 128 : (tj + 1) * 128],
                        rhs=Wv_sb[:, ck, :],
                        start=(ck == 0),
                        stop=(ck == CK - 1),
                    )
                nc.scalar.copy(v_sb[:, tj, :], v_psum[:, 0:DV])

            pvns = {}

            def finish_block(ib):
                # zacc holds the four (head, chunk-parity) Z partials in
                # 32-partition bands; one fp32 sel-matmul sums the parities
                # and broadcasts each head's Z to its 64 pv partitions.
                pv = pvs.pop(ib)
                zacc = zbs.pop(ib)
                zsb = work.tile([128, IB], f32, tag="zsb", name=f"zsb_{ib}")
                nc.scalar.copy(zsb, zacc)
                zfull = zb_ps.tile([128, IB], f32, tag="zb", name=f"zfull_{ib}")
                nc.tensor.matmul(zfull, lhsT=sel_sb, rhs=zsb, start=True, stop=True)
                zr = work.tile([128, IB], f32, tag="zr", name=f"zr_{ib}")
                nc.vector.reciprocal_approx_fast(zr, zfull)
                pvn = work.tile([128, IB], bf16, tag="pvn", name=f"pvn_{ib}")
                nc.vector.tensor_mul(pvn, pv, zr)
                pvns[ib] = pvn

            def fo_step(ib, isub):
                pvn = pvns[ib]
                fo = pv_ps.tile([128, C], f32, tag="pv", name=f"fo_{ib}_{isub}")
                nc.tensor.matmul(
                    fo,
                    lhsT=pvn[:, isub * 128 : (isub + 1) * 128],
                    rhs=Wu_sb,
                    start=True,
                    stop=True,
                )
                fo_sb = outsb.tile([128, C], f32, tag="fo", name=f"fs_{ib}_{isub}")
                nc.scalar.copy(fo_sb, fo)
                t0 = ib * IB + isub * 128
                nc.sync.dma_start(out_d[t0 : t0 + 128, :], fo_sb)
                if isub == IB // 128 - 1:
                    del pvns[ib]

            # --- paired-chunk PV/Z passes: every pass is a 4x32 col-tiled
            # group so the PE runs 4 concurrent streams per pass and the Z
            # broadcast shares passes with PV instead of owning its own.
            # zacc bands: [0:32] Zh0(odd tj), [32:64] Zh1(odd),
            #             [64:96] Zh0(even),  [96:128] Zh1(even)
            def pass_pv(ib, m):
                # all-PV pass for even chunk m
                if m == 0:
                    pvs[ib] = pv_ps.tile([128, IB], f32, tag="pv", name=f"pv_{ib}")
                pv = pvs[ib]
                pt = pts[ib * TJ + m]
                for c in range(4):
                    nc.tensor.matmul(
                        pv[32 * c : 32 * c + 32, :],
                        lhsT=v_sb[:, m, 32 * c : 32 * c + 32],
                        rhs=pt[:, c // 2, :],
                        start=(m == 0),
                        stop=False,
                        tile_position=(0, 32 * c),
                    )

            def pass_zpv(ib, m):
                # Z(m) at cols 64/96 + PV h0(m+1) at cols 0/32
                if m == 0:
                    zbs[ib] = zb_ps.tile([128, IB], f32, tag="zb", name=f"zb_{ib}")
                pv = pvs[ib]
                zacc = zbs[ib]
                ptm = pts.pop(ib * TJ + m)
                ptm1 = pts[ib * TJ + m + 1]
                nc.tensor.matmul(
                    zacc[64:96, :],
                    lhsT=ones_sb[:, 0:32],
                    rhs=ptm[:, 0, :],
                    start=(m == 0),
                    stop=(m == TJ - 2),
                    tile_position=(0, 64),
                )
                nc.tensor.matmul(
                    zacc[96:128, :],
                    lhsT=ones_sb[:, 0:32],
                    rhs=ptm[:, 1, :],
                    start=(m == 0),
                    stop=(m == TJ - 2),
                    tile_position=(0, 96),
                )
                for c in range(2):
                    nc.tensor.matmul(
                        pv[32 * c : 32 * c + 32, :],
                        lhsT=v_sb[:, m + 1, 32 * c : 32 * c + 32],
                        rhs=ptm1[:, 0, :],
                        start=False,
                        stop=(m == TJ - 2),
                        tile_position=(0, 32 * c),
                    )

            def pass_pvz(ib, m):
                # PV h1(m+1) at cols 64/96 + Z(m+1) at cols 0/32
                pv = pvs[ib]
                zacc = zbs[ib]
                ptm1 = pts.pop(ib * TJ + m + 1)
                for c in range(2, 4):
                    nc.tensor.matmul(
                        pv[32 * c : 32 * c + 32, :],
                        lhsT=v_sb[:, m + 1, 32 * c : 32 * c + 32],
                        rhs=ptm1[:, 1, :],
                        start=False,
                        stop=(m == TJ - 2),
                        tile_position=(0, 32 * c),
                    )
                nc.tensor.matmul(
                    zacc[0:32, :],
                    lhsT=ones_sb[:, 0:32],
                    rhs=ptm1[:, 0, :],
                    start=(m == 0),
                    stop=(m == TJ - 2),
                    tile_position=(0, 0),
                )
                nc.tensor.matmul(
                    zacc[32:64, :],
                    lhsT=ones_sb[:, 0:32],
                    rhs=ptm1[:, 1, :],
                    start=(m == 0),
                    stop=(m == TJ - 2),
                    tile_position=(0, 32),
                )
                if m == TJ - 2:
                    finish_block(ib)

            # pass schedule: for each pair (m, m+1), pass_pv at slot m+3,
            # pass_zpv at m+4, pass_pvz at m+5
            sched = {}
            for ib in range(NIB):
                for m in range(0, TJ, 2):
                    base = ib * TJ + m
                    sched.setdefault(base + 3, []).append(("pv", ib, m))
                    sched.setdefault(base + 4, []).append(("zpv", ib, m))
                    sched.setdefault(base + 5, []).append(("pvz", ib, m))
            PASS_FN = {}

            qproj_state = {}

            def q_proj_step(ib, step):
                # one K=128 partial of next block's qT projection; the psum
                # group stays open across several chunks so the PE absorbs
                # it in its per-chunk slack instead of one big bubble
                ts = ib + 1
                if step == 0:
                    qproj_state[ts] = pv_ps.tile(
                        [128, 512], f32, tag="pv", name=f"qp_{ts}"
                    )
                p_ps = qproj_state[ts]
                sl = slice(ts * 512, (ts + 1) * 512)
                nc.tensor.matmul(
                    p_ps,
                    lhsT=Wq_sb[:, step, :],
                    rhs=condT_sb[:, step, sl],
                    start=(step == 0),
                    stop=(step == CK - 1),
                )
                if step == CK - 1:
                    nc.vector.tensor_copy(qT_sb[:, sl], p_ps)
                    del qproj_state[ts]

            PASS_FN.update({"pv": pass_pv, "zpv": pass_zpv, "pvz": pass_pvz})

            N = NIB * TJ
            for n in range(N):
                ib, tj = divmod(n, TJ)
                if n < TJ and n % 4 == 0:
                    qk_proj(n // 4, Wk_sb, kT_sb)  # kT slice just ahead of use
                if n == 0:
                    qk_proj(0, Wq_sb, qT_sb)
                if ib + 1 < NIB and tj in (16, 18, 20, 22):
                    q_proj_step(ib, (tj - 16) // 2)
                if ib > 0 and tj in (4, 6, 8, 10):
                    fo_step(ib - 1, (tj - 4) // 2)
                if n < TJ:
                    v_proj_chunk(n)
                isl = slice(ib * IB, (ib + 1) * IB)
                jsl = slice(tj * 128, (tj + 1) * 128)
                st = stage_ps.tile([128, 2, 512], f32, tag="stage", name=f"st_{n}")
                # scores S^T[j, i] per head; K=64 row-packed (h0 rows 0-63,
                # h1 rows 64-127) -> concurrent on the PE
                nc.tensor.matmul(
                    st[:, 0, :],
                    lhsT=kT_sb[0:64, jsl],
                    rhs=qT_sb[0:64, isl],
                    start=True,
                    stop=True,
                )
                nc.tensor.matmul(
                    st[:, 1, :],
                    lhsT=kT_sb[64:128, jsl],
                    rhs=qT_sb[64:128, isl],
                    start=True,
                    stop=True,
                )
                pt = pt_pool.tile([128, 2, 512], bf16, tag="pt", name=f"pt_{n}")
                I0 = EXP_SPLIT_I
                nc.scalar.activation(
                    pt[:, :, 0:I0], st[:, :, 0:I0], Exp, bias=ebias_sb[:, :], scale=1.0
                )
                # Schraudolph exp on the DVE: int16 bits written into the
                # bf16 tile ARE the bf16 of ~exp(st - 8)
                nc.vector._custom_dve(
                    exp_op,
                    out=pt[:, :, I0:512].bitcast(i16),
                    in0=st[:, :, I0:512],
                    s0=EXP_C0,
                    s1=EXP_C1,
                    imm2=EXP_CLAMP,
                )
                pts[n] = pt
                for kind, pib, pm in sched.get(n, []):
                    PASS_FN[kind](pib, pm)
            for n in range(N, N + 6):
                for kind, pib, pm in sched.get(n, []):
                    PASS_FN[kind](pib, pm)
            for isub in range(IB // 128):
                fo_step(NIB - 1, isub)

    nc.compile()
    return nc


def _get_nc():
    global _BUILT
    if _BUILT is None:
        _BUILT = _build_nc()
    return _BUILT


def kernel(x, condition, W_qk, W_v, W_u, b_u):
    from concourse.bass_utils import run_bass_kernel_spmd

    bf = ml_dtypes.bfloat16
    x = np.asarray(x, dtype=np.float32)
    condition = np.asarray(condition, dtype=np.float32)
    W_qk = np.asarray(W_qk, dtype=np.float32)
    W_v = np.asarray(W_v, dtype=np.float32)
    W_u = np.asarray(W_u, dtype=np.float32)
    b_u = np.asarray(b_u, dtype=np.float32)

    Wq = (W_qk[:, : H * DH] * SCALE).astype(bf)
    Wk = W_qk[:, H * DH :].astype(bf)
    Wv = W_v.astype(bf)
    Wu = W_u.astype(bf)
    condT = np.ascontiguousarray(condition.transpose(0, 2, 1)).astype(bf)
    xT = np.ascontiguousarray(x.transpose(0, 2, 1)).astype(bf)

    # Z parity-combine/broadcast selector: zfull[p] = sum_k sel[k,p]*zacc[k]
    sel = np.zeros((128, 128), dtype=np.float32)
    for p in range(64):
        sel[p % 32, p] = 1.0
        sel[64 + p % 32, p] = 1.0
    for p in range(64, 128):
        sel[32 + p % 32, p] = 1.0
        sel[96 + p % 32, p] = 1.0

    in_maps = []
    for core in range(NCORES):
        b = core // 4
        hp = core % 4
        ds = slice(hp * DV, (hp + 1) * DV)
        in_maps.append(
            {
                "condT": condT[b],
                "xT": xT[b],
                "Wq": np.ascontiguousarray(Wq[:, ds]),
                "Wk": np.ascontiguousarray(Wk[:, ds]),
                "Wv": np.ascontiguousarray(Wv[:, ds]),
                "Wu": np.ascontiguousarray(Wu[ds, :]),
                "sel": sel,
            }
        )

    nc = _get_nc()
    res = run_bass_kernel_spmd(nc, in_maps, core_ids=list(range(NCORES)))
    results = res.results

    out = np.zeros((B, T, C), dtype=np.float32)
    for core in range(NCORES):
        out[core // 4] += results[core]["out"]
    out += b_u
    return out

